# revision 11
# baseline (speedup 1.0000x reference)
"""AE (associative embedding) push/pull loss on 8 Trainium2 NeuronCores.

Data-parallel over the batch: core c handles images [4c, 4c+4). Per core the
kernel gathers only the visible (person, joint) tag values out of the
on-device 4x1114112 tag shard with indirect (SWDGE) DMAs -- the visible
slots are host-compacted into C columns of 128 so each gather instruction
moves 128 scattered f32 elements. Per-person sums of t and t^2 come from C
accumulating PE matmuls against host-built person-indicator matrices
(pipelined: square on ScalarE + matmul on PE run behind each gather). The
push loss builds the block-diagonal 120x120 pairwise exp(-(mu_i-mu_j)^2)
tile via a PE transpose, one ScalarE Square (bias=-mu) and one Exp, then a
masked row-reduce and a final PE matmul against a per-image indicator.

Host-side work is index/mask preparation only (no tag data is touched):
compacted gather indices, per-person 1/cnt and validity, indicator
matrices, per-image n and denominators. All tag-data movement and
arithmetic happens on-device. Raw bacc (no TileContext): per-engine chain
semaphores serialize same-engine RAW hazards and double as cross-engine
handshakes; input DMAs are hoisted into the preamble region.
"""

import math
from contextlib import ExitStack

import numpy as np

try:
    import concourse  # noqa: F401
except ImportError:
    import sys

    sys.path.insert(0, "/opt/trn_rl_repo")

from concourse import bacc, bass, mybir
from concourse.bass_utils import run_bass_kernel_spmd

N, M, K, KHW = 32, 30, 17, 1114112
NCORES = 8
IPC = N // NCORES
P = 128
PPI = IPC * M

f32 = mybir.dt.float32
i32 = mybir.dt.int32
Alu = mybir.AluOpType
Act = mybir.ActivationFunctionType


BIG = 1.0e4  # additive mask magnitude: exp(-(BIG+d)^2) underflows to 0


def _consts_layout(C):
    # cols: [lhsT_0..lhsT_{C-1} | rc | negrc | pv | pvnb | nim | rdenom |
    #        rnim | bmask | binds | ident]
    RC0 = 128 * C
    NRC0 = RC0 + 1
    PV0 = NRC0 + 1
    PVN0 = PV0 + 1
    NI0 = PVN0 + 1
    RD0 = NI0 + 1
    RN0 = RD0 + 1
    BM0 = RN0 + 1
    BI0 = BM0 + P
    ID0 = BI0 + IPC
    CC = ID0 + P
    return RC0, NRC0, PV0, PVN0, NI0, RD0, RN0, BM0, BI0, ID0, CC


def _build_nc(C, R=P):
    # R: partition rows gathered for the last column (rest is padding,
    # kept defined by a TT memset).
    RC0, NRC0, PV0, PVN0, NI0, RD0, RN0, BM0, BI0, ID0, CC = _consts_layout(C)
    nc = bacc.Bacc(
        "TRN2",
        target_bir_lowering=False,
        debug=False,
        enable_asserts=False,
        num_devices=NCORES,
        detect_race_conditions=False,
    )
    tags_d = nc.declare_dram_parameter("tags", [IPC * KHW, 1], f32, isOutput=False)
    gidx_d = nc.declare_dram_parameter("gidx", [P, C], i32, isOutput=False)
    consts_d = nc.declare_dram_parameter("consts", [P, CC], f32, isOutput=False)
    out_d = nc.declare_dram_parameter("out", [IPC, 2], f32, isOutput=True)

    ctx = ExitStack()
    g_sem = ctx.enter_context(nc.semaphore("g_sem"))
    c_sem = ctx.enter_context(nc.semaphore("c_sem"))
    o_sem = ctx.enter_context(nc.semaphore("o_sem"))
    vc = ctx.enter_context(nc.semaphore("vc_sem"))
    tc = ctx.enter_context(nc.semaphore("tc_sem"))
    sc = ctx.enter_context(nc.semaphore("sc_sem"))
    tcol = [ctx.enter_context(nc.semaphore(f"tcol{c}")) for c in range(C)]
    d_sem = ctx.enter_context(nc.semaphore("d_sem"))
    dd_sem = ctx.enter_context(nc.semaphore("dd_sem"))

    gidx_sb = ctx.enter_context(nc.sbuf_tensor("gidx_sb", [P, C], i32))
    c_sb = ctx.enter_context(nc.sbuf_tensor("c_sb", [P, CC], f32))
    TT = ctx.enter_context(nc.sbuf_tensor("TT", [P, 2 * C], f32))
    mu = ctx.enter_context(nc.sbuf_tensor("mu", [P, 1], f32))
    nmu = ctx.enter_context(nc.sbuf_tensor("nmu", [P, 1], f32))
    t2s = ctx.enter_context(nc.sbuf_tensor("t2s", [P, 1], f32))
    negp = ctx.enter_context(nc.sbuf_tensor("negp", [P, 1], f32))
    X = ctx.enter_context(nc.sbuf_tensor("X", [P, 2], f32))
    d2 = ctx.enter_context(nc.sbuf_tensor("d2", [P, P], f32))
    e = ctx.enter_context(nc.sbuf_tensor("e", [P, P], f32))
    pm = ctx.enter_context(nc.sbuf_tensor("pm", [P, P], f32))
    me = ctx.enter_context(nc.sbuf_tensor("me", [P, P], f32))
    res = ctx.enter_context(nc.sbuf_tensor("res", [IPC, 2], f32))
    warm = ctx.enter_context(nc.sbuf_tensor("warm", [1, 1], f32))
    didx = ctx.enter_context(nc.sbuf_tensor("didx", [16, 1], i32))
    dt_sb = ctx.enter_context(nc.sbuf_tensor("dt_sb", [16, 1], f32))
    muT_t = ctx.enter_context(nc.psum_tensor("muT", [P, 512], f32))
    pvT_t = ctx.enter_context(nc.psum_tensor("pvT", [P, 512], f32))
    fmu_t = ctx.enter_context(nc.psum_tensor("fmup", [P, 512], f32))
    fin_t = ctx.enter_context(nc.psum_tensor("fin", [IPC, 512], f32))

    rc_ap = c_sb.ap()[:, RC0 : RC0 + 1]
    negrc_ap = c_sb.ap()[:, NRC0 : NRC0 + 1]
    pv_ap = c_sb.ap()[:, PV0 : PV0 + 1]
    pvnb_ap = c_sb.ap()[:, PVN0 : PVN0 + 1]
    nim_ap = c_sb.ap()[0:IPC, NI0 : NI0 + 1]
    rd_ap = c_sb.ap()[0:IPC, RD0 : RD0 + 1]
    rn_ap = c_sb.ap()[0:IPC, RN0 : RN0 + 1]
    bmask = c_sb.ap()[:, BM0 : BM0 + P]
    binds = c_sb.ap()[:, BI0 : BI0 + IPC]
    ident = c_sb.ap()[:, ID0 : ID0 + P]
    muT = muT_t.ap()[:, :P]
    pvT = pvT_t.ap()[:, :P]
    fmup = fmu_t.ap()[:, :2]
    fin = fin_t.ap()[:, :2]

    vn = {"n": 0}

    def nxt():
        vn["n"] += 1
        return vn["n"]

    V_BIGM = 3
    V_MU = 4
    V_NMU = 5
    V_X1 = 8
    V_RES = 10
    S_SQ = lambda c: 1 + c  # scalar-chain: column-c square done
    S_D2 = C + 1
    S_EXP = C + 2

    T_PVT = 1
    T_MM = lambda c: 3 + c
    T_MUT = C + 3
    T_FIN = C + 4

    with nc.Block(no_gpsimd_drain=True) as block:

        @block.sync
        def _(sync):
            sync.dma_start(out=gidx_sb.ap(), in_=gidx_d[:]).then_inc(g_sem, 16)
            sync.dma_start(out=c_sb.ap(), in_=consts_d[:]).then_inc(c_sem, 16)
            sync.wait_ge(vc, V_RES)
            sync.dma_start(out=out_d[:], in_=res.ap()).then_inc(o_sem, 16)
            sync.wait_ge(o_sem, 16)

        @block.gpsimd
        def _(gpsimd):
            # warm the indirect-DMA ucode path while the gidx DMA is in flight
            gpsimd.memset(didx.ap(), 0).then_inc(d_sem, 1)
            gpsimd.wait_ge(d_sem, 1)
            gpsimd.indirect_dma_start(
                out=dt_sb.ap(),
                out_offset=None,
                in_=tags_d[:],
                in_offset=bass.IndirectOffsetOnAxis(ap=didx.ap(), axis=0),
            ).then_inc(dd_sem, 16)
            gpsimd.wait_ge(g_sem, 16)
            for c in range(C):
                rows = P if c < C - 1 else R
                if c == C - 1 and R < P:
                    gpsimd.wait_ge(vc, 1)  # TT memset done
                gpsimd.indirect_dma_start(
                    out=TT.ap()[0:rows, c : c + 1],
                    out_offset=None,
                    in_=tags_d[:],
                    in_offset=bass.IndirectOffsetOnAxis(
                        ap=gidx_sb.ap()[0:rows, c : c + 1], axis=0
                    ),
                ).then_inc(tcol[c], 16)

        @block.vector
        def _(vector):
            def chain(instr):
                instr.then_inc(vc, 1)
                return nxt()

            def W():
                vector.wait_ge(vc, vn["n"])

            # 1: keep the unwritten tail of the last gather column defined
            chain(vector.memset(TT.ap()[:, C - 1 : C], 0.0))
            # 2-3: additive pair mask BIGM = BIG * (1 - pv_p*pv_q*blk)
            vector.wait_ge(tc, T_PVT)
            chain(vector.scalar_tensor_tensor(
                out=pm.ap(), in0=pvT, scalar=pvnb_ap, in1=bmask,
                op0=Alu.mult, op1=Alu.mult))
            W()
            i = chain(vector.tensor_scalar(
                out=me.ap(), in0=pm.ap(), scalar1=BIG, scalar2=None,
                op0=Alu.add))
            assert i == V_BIGM
            # per-person stats straight from PSUM
            vector.wait_ge(tc, T_MM(C - 1))
            W()
            i = chain(vector.tensor_mul(mu.ap(), fmup[:, 0:1], rc_ap))
            assert i == V_MU
            W()
            i = chain(vector.tensor_mul(nmu.ap(), fmup[:, 0:1], negrc_ap))
            assert i == V_NMU
            W()
            chain(vector.tensor_copy(t2s.ap(), fmup[:, 1:2]))
            W()
            chain(vector.scalar_tensor_tensor(
                out=negp.ap(), in0=fmup[:, 0:1], scalar=mu.ap(), in1=t2s.ap(),
                op0=Alu.mult, op1=Alu.subtract))
            W()
            i = chain(vector.tensor_scalar(
                out=X.ap()[:, 1:2], in0=negp.ap(), scalar1=rc_ap, scalar2=-1.0,
                op0=Alu.mult, op1=Alu.mult))
            assert i == V_X1
            # epilogue straight from fin PSUM (rd already includes the 0.5)
            vector.wait_ge(tc, T_FIN)
            W()
            chain(vector.tensor_scalar(
                out=res.ap()[:, 0:1], in0=fin[:, 0:1], scalar1=nim_ap,
                scalar2=rd_ap, op0=Alu.subtract, op1=Alu.mult))
            W()
            i = chain(vector.tensor_mul(res.ap()[:, 1:2], fin[:, 1:2], rn_ap))
            assert i == V_RES

        @block.tensor
        def _(tensor):
            tensor.wait_ge(c_sem, 16)
            tensor.transpose(
                out=pvT, in_=pv_ap.to_broadcast([P, P]), identity=ident
            ).then_inc(tc, 1)
            # preload muT's PSUM bank with the (symmetric) additive mask;
            # the mu transpose later accumulates on top of it
            tensor.wait_ge(vc, V_BIGM)
            tensor.matmul(
                out=muT, lhsT=me.ap(), rhs=ident, is_transpose=True,
                start=True, stop=False, skip_group_check=True,
            ).then_inc(tc, 1)
            for c in range(C):
                tensor.wait_ge(sc, S_SQ(c))
                tensor.matmul(
                    out=fmup,
                    lhsT=c_sb.ap()[:, 128 * c : 128 * (c + 1)],
                    rhs=TT.ap()[:, c : 2 * C : C],
                    start=(c == 0),
                    stop=(c == C - 1),
                    skip_group_check=True,
                ).then_inc(tc, 1)
            tensor.wait_ge(vc, V_MU)
            tensor.matmul(
                out=muT, lhsT=mu.ap().to_broadcast([P, P]), rhs=ident,
                is_transpose=True, start=False, stop=True,
                skip_group_check=True,
            ).then_inc(tc, 1)
            tensor.wait_ge(sc, S_EXP)
            tensor.wait_ge(vc, V_X1)
            tensor.matmul(
                out=fin, lhsT=binds, rhs=X.ap(), start=True, stop=True
            ).then_inc(tc, 1)

        @block.scalar
        def _(scalar):
            scalar.wait_ge(c_sem, 16)
            scalar.activation(warm.ap(), c_sb.ap()[0:1, 0:1], Act.Exp)
            for c in range(C):
                scalar.wait_ge(tcol[c], 16)
                scalar.activation(
                    TT.ap()[:, C + c : C + c + 1],
                    TT.ap()[:, c : c + 1],
                    Act.Square,
                ).then_inc(sc, 1)
            scalar.wait_ge(tc, T_MUT)
            scalar.wait_ge(vc, V_NMU)
            scalar.activation(
                d2.ap(), muT, Act.Square, bias=nmu.ap()
            ).then_inc(sc, 1)
            scalar.wait_ge(sc, S_D2)  # d2 write visible before Exp reads it
            scalar.activation(
                e.ap(), d2.ap(), Act.Exp, scale=-1.0, accum_out=X.ap()[:, 0:1]
            ).then_inc(sc, 1)

    ctx.close()
    _hoist_input_dmas(nc)
    nc.compile()
    return nc


def _hoist_input_dmas(nc):
    """Move the two input DMAs from sync's body block into the entry block,
    right after sync's TPB-base preamble, so the gidx/consts loads overlap
    the remaining engine boot instead of waiting for the entry barrier."""
    f = nc.m.functions[0]
    b0 = f.blocks[0]
    bsync = next(b for b in f.blocks if "_SP_" in b.name)
    assert type(bsync.instructions[0]).__name__ == "InstDMACopy"
    assert type(bsync.instructions[1]).__name__ == "InstDMACopy"
    dmas = [bsync.instructions[0], bsync.instructions[1]]
    rest = list(bsync.instructions)[2:]
    bsync.set_instructions_from_list(rest) if hasattr(
        bsync, "set_instructions_from_list"
    ) else None
    if not hasattr(bsync, "set_instructions_from_list"):
        del bsync.instructions[0]
        del bsync.instructions[0]
    sp = mybir.EngineType.SP
    entry = list(b0.instructions)
    pos = min(i for i, inst in enumerate(entry) if inst.engine == sp)
    b0.instructions[pos:pos] = dmas


def _prepare_in_maps_c(tags, joints, C):
    RC0, NRC0, PV0, PVN0, NI0, RD0, RN0, BM0, BI0, ID0, CC = _consts_layout(C)
    tags = np.ascontiguousarray(tags, dtype=np.float32).reshape(N, KHW)
    joints = np.asarray(joints)
    idx = joints[..., 0].astype(np.int64)
    visb = joints[..., 1] > 0

    blkid = np.repeat(np.arange(IPC), M)
    bmask = np.zeros((P, P), np.float32)
    bmask[:PPI, :PPI] = (blkid[:, None] == blkid[None, :]).astype(np.float32)
    binds = np.zeros((P, IPC), np.float32)
    for i in range(IPC):
        binds[i * M : (i + 1) * M, i] = 1.0
    ident = np.eye(P, dtype=np.float32)

    in_maps = []
    for cidx in range(NCORES):
        sl = slice(cidx * IPC, (cidx + 1) * IPC)
        vb = visb[sl].reshape(PPI, K)
        cnt = vb.sum(axis=1).astype(np.float32)
        gfull = (
            np.arange(IPC, dtype=np.int64)[:, None, None] * KHW + idx[sl]
        ).reshape(PPI, K)
        pp, kk = np.nonzero(vb)
        vals = gfull[pp, kk]
        S = len(pp)
        if S > 128 * C:
            return None
        s = np.arange(S)
        rows, cols = s % P, s // P
        gidxC = np.zeros((P, C), np.int32)
        gidxC[rows, cols] = vals.astype(np.int32)
        lhs = np.zeros((P, C * P), np.float32)
        lhs[rows, cols * P + pp] = 1.0

        pv = (cnt > 0).astype(np.float32)
        nim = pv.reshape(IPC, M).sum(axis=1)  # [IPC]
        consts = np.zeros((P, CC), np.float32)
        consts[:, :RC0] = lhs
        consts[:PPI, RC0] = 1.0 / np.maximum(cnt, 1.0)
        consts[PPI:, RC0] = 1.0
        consts[:, NRC0] = -consts[:, RC0]
        consts[:PPI, PV0] = pv
        consts[:PPI, PVN0] = -BIG * pv
        consts[:IPC, NI0] = nim
        consts[:IPC, RD0] = (0.5 / np.maximum((nim - 1.0) * nim, 1.0)).astype(
            np.float32
        )
        consts[:IPC, RN0] = (1.0 / np.maximum(nim, 1.0)).astype(np.float32)
        consts[:, BM0 : BM0 + P] = bmask
        consts[:, BI0 : BI0 + IPC] = binds
        consts[:, ID0 : ID0 + P] = ident
        in_maps.append(
            {
                "tags": tags[sl].reshape(IPC * KHW, 1),
                "gidx": gidxC,
                "consts": consts,
            }
        )
    return in_maps


_CACHE: dict = {}


def _get_nc(C, R):
    if (C, R) not in _CACHE:
        _CACHE[(C, R)] = _build_nc(C, R)
    return _CACHE[(C, R)]


def _needed_CR(joints):
    visb = np.asarray(joints)[..., 1] > 0
    smax = 0
    for c in range(NCORES):
        S = int(visb[c * IPC : (c + 1) * IPC].sum())
        smax = max(smax, S)
    C = min(max(1, math.ceil(smax / P)), K)
    # last-column fill, rounded up to 32 rows to bound NEFF-cache variants
    R = smax - (C - 1) * P
    R = min(P, max(32, 32 * math.ceil(R / 32)))
    return C, R


def _run(tags, joints, trace=False, **kwargs):
    tags = np.asarray(tags)
    joints = np.asarray(joints)
    C, R = _needed_CR(joints)
    nc = _get_nc(C, R)
    in_maps = _prepare_in_maps_c(tags, joints, C)
    assert in_maps is not None
    res = run_bass_kernel_spmd(
        nc, in_maps, core_ids=list(range(NCORES)), trace=trace, **kwargs
    )
    outs = np.concatenate(
        [np.asarray(res.results[i]["out"]) for i in range(NCORES)], axis=0
    )  # [N, 2]
    pushes = np.ascontiguousarray(outs[:, 0], dtype=np.float32)
    pulls = np.ascontiguousarray(outs[:, 1], dtype=np.float32)
    return (pushes, pulls), res


def kernel(tags, joints):
    (pushes, pulls), _ = _run(tags, joints, trace=False)
    return pushes, pulls


# revision 12
# speedup vs baseline: 1.1661x; 1.1661x over previous
"""AE (associative embedding) push/pull loss on 8 Trainium2 NeuronCores.

Data-parallel over the batch: core c handles images [4c, 4c+4). Per core the
kernel gathers only the visible (person, joint) tag values out of the
on-device 4x1114112 tag shard with indirect (SWDGE) DMAs -- the visible
slots are host-compacted into C columns of 128 so each gather instruction
moves 128 scattered f32 elements. Per-person sums of t and t^2 come from C
accumulating PE matmuls against host-built person-indicator matrices
(pipelined: square on ScalarE + matmul on PE run behind each gather). The
push loss builds the block-diagonal 120x120 pairwise exp(-(mu_i-mu_j)^2)
tile via a PE transpose, one ScalarE Square (bias=-mu) and one Exp, then a
masked row-reduce and a final PE matmul against a per-image indicator.

Host-side work is index/mask preparation only (no tag data is touched):
compacted gather indices, per-person 1/cnt and validity, indicator
matrices, per-image n and denominators. All tag-data movement and
arithmetic happens on-device. Raw bacc (no TileContext): per-engine chain
semaphores serialize same-engine RAW hazards and double as cross-engine
handshakes; input DMAs are hoisted into the preamble region.
"""

import math
from contextlib import ExitStack

import numpy as np

try:
    import concourse  # noqa: F401
except ImportError:
    import sys

    sys.path.insert(0, "/opt/trn_rl_repo")

from concourse import bacc, bass, mybir
from concourse.bass_utils import run_bass_kernel_spmd

N, M, K, KHW = 32, 30, 17, 1114112
NCORES = 8
IPC = N // NCORES
P = 128
PPI = IPC * M

f32 = mybir.dt.float32
i32 = mybir.dt.int32
Alu = mybir.AluOpType
Act = mybir.ActivationFunctionType


BIG = 1.0e4  # additive mask magnitude: exp(-(BIG+d)^2) underflows to 0


def _consts_layout(C):
    # cols: [lhsT_0..lhsT_{C-1} | rc | negrc | pv | pvnb | nim | rdenom |
    #        rnim | bmask | binds | ident]
    RC0 = 128 * C
    NRC0 = RC0 + 1
    PV0 = NRC0 + 1
    PVN0 = PV0 + 1
    NI0 = PVN0 + 1
    RD0 = NI0 + 1
    RN0 = RD0 + 1
    BM0 = RN0 + 1
    BI0 = BM0 + P
    ID0 = BI0 + IPC
    CC = ID0 + P
    return RC0, NRC0, PV0, PVN0, NI0, RD0, RN0, BM0, BI0, ID0, CC


def _build_nc(C, R=P):
    # R: partition rows gathered for the last column (rest is padding,
    # kept defined by a TT memset).
    RC0, NRC0, PV0, PVN0, NI0, RD0, RN0, BM0, BI0, ID0, CC = _consts_layout(C)
    nc = bacc.Bacc(
        "TRN2",
        target_bir_lowering=False,
        debug=False,
        enable_asserts=False,
        num_devices=NCORES,
        detect_race_conditions=False,
    )
    tags_d = nc.declare_dram_parameter("tags", [IPC * KHW, 1], f32, isOutput=False)
    gidx_d = nc.declare_dram_parameter("gidx", [P, C], i32, isOutput=False)
    consts_d = nc.declare_dram_parameter("consts", [P, CC], f32, isOutput=False)
    out_d = nc.declare_dram_parameter("out", [IPC, 2], f32, isOutput=True)

    ctx = ExitStack()
    g_sem = ctx.enter_context(nc.semaphore("g_sem"))
    c_sem = ctx.enter_context(nc.semaphore("c_sem"))
    o_sem = ctx.enter_context(nc.semaphore("o_sem"))
    vc = ctx.enter_context(nc.semaphore("vc_sem"))
    tc = ctx.enter_context(nc.semaphore("tc_sem"))
    sc = ctx.enter_context(nc.semaphore("sc_sem"))
    tcol = [ctx.enter_context(nc.semaphore(f"tcol{c}")) for c in range(C)]
    d_sem = ctx.enter_context(nc.semaphore("d_sem"))
    dd_sem = ctx.enter_context(nc.semaphore("dd_sem"))

    gidx_sb = ctx.enter_context(nc.sbuf_tensor("gidx_sb", [P, C], i32))
    c_sb = ctx.enter_context(nc.sbuf_tensor("c_sb", [P, CC], f32))
    TT = ctx.enter_context(nc.sbuf_tensor("TT", [P, 2 * C], f32))
    mu = ctx.enter_context(nc.sbuf_tensor("mu", [P, 1], f32))
    nmu = ctx.enter_context(nc.sbuf_tensor("nmu", [P, 1], f32))
    t2s = ctx.enter_context(nc.sbuf_tensor("t2s", [P, 1], f32))
    negp = ctx.enter_context(nc.sbuf_tensor("negp", [P, 1], f32))
    X = ctx.enter_context(nc.sbuf_tensor("X", [P, 2], f32))
    d2 = ctx.enter_context(nc.sbuf_tensor("d2", [P, P], f32))
    e = ctx.enter_context(nc.sbuf_tensor("e", [P, P], f32))
    pm = ctx.enter_context(nc.sbuf_tensor("pm", [P, P], f32))
    me = ctx.enter_context(nc.sbuf_tensor("me", [P, P], f32))
    res = ctx.enter_context(nc.sbuf_tensor("res", [IPC, 2], f32))
    warm = ctx.enter_context(nc.sbuf_tensor("warm", [1, 1], f32))
    didx = ctx.enter_context(nc.sbuf_tensor("didx", [16, 1], i32))
    dt_sb = ctx.enter_context(nc.sbuf_tensor("dt_sb", [16, 1], f32))
    muT_t = ctx.enter_context(nc.psum_tensor("muT", [P, 512], f32))
    pvT_t = ctx.enter_context(nc.psum_tensor("pvT", [P, 512], f32))
    fmu_t = ctx.enter_context(nc.psum_tensor("fmup", [P, 512], f32))
    fin_t = ctx.enter_context(nc.psum_tensor("fin", [IPC, 512], f32))

    rc_ap = c_sb.ap()[:, RC0 : RC0 + 1]
    negrc_ap = c_sb.ap()[:, NRC0 : NRC0 + 1]
    pv_ap = c_sb.ap()[:, PV0 : PV0 + 1]
    pvnb_ap = c_sb.ap()[:, PVN0 : PVN0 + 1]
    nim_ap = c_sb.ap()[0:IPC, NI0 : NI0 + 1]
    rd_ap = c_sb.ap()[0:IPC, RD0 : RD0 + 1]
    rn_ap = c_sb.ap()[0:IPC, RN0 : RN0 + 1]
    bmask = c_sb.ap()[:, BM0 : BM0 + P]
    binds = c_sb.ap()[:, BI0 : BI0 + IPC]
    ident = c_sb.ap()[:, ID0 : ID0 + P]
    muT = muT_t.ap()[:, :P]
    pvT = pvT_t.ap()[:, :P]
    fmup = fmu_t.ap()[:, :2]
    fin = fin_t.ap()[:, :2]

    vn = {"n": 0}

    def nxt():
        vn["n"] += 1
        return vn["n"]

    V_BIGM = 3
    V_MU = 4
    V_NMU = 5
    V_X1 = 8
    V_RES = 9
    S_SQ = lambda c: 1 + c  # scalar-chain: column-c square done
    S_D2 = C + 1
    S_EXP = C + 2
    S_RES1 = C + 3

    T_PVT = 1
    T_MM = lambda c: 3 + c
    T_MUT = C + 3
    T_FIN = C + 4

    with nc.Block(no_gpsimd_drain=True) as block:

        @block.sync
        def _(sync):
            sync.dma_start(out=gidx_sb.ap(), in_=gidx_d[:]).then_inc(g_sem, 16)
            sync.dma_start(out=c_sb.ap(), in_=consts_d[:]).then_inc(c_sem, 16)
            sync.wait_ge(vc, V_RES)
            sync.wait_ge(sc, S_RES1)
            sync.dma_start(out=out_d[:], in_=res.ap()).then_inc(o_sem, 16)
            sync.wait_ge(o_sem, 16)

        @block.gpsimd
        def _(gpsimd):
            # warm the indirect-DMA ucode path while the gidx DMA is in flight
            gpsimd.memset(didx.ap(), 0).then_inc(d_sem, 1)
            gpsimd.wait_ge(d_sem, 1)
            gpsimd.indirect_dma_start(
                out=dt_sb.ap(),
                out_offset=None,
                in_=tags_d[:],
                in_offset=bass.IndirectOffsetOnAxis(ap=didx.ap(), axis=0),
            ).then_inc(dd_sem, 16)
            gpsimd.wait_ge(g_sem, 16)
            for c in range(C):
                rows = P if c < C - 1 else R
                if c == C - 1 and R < P:
                    gpsimd.wait_ge(vc, 1)  # TT memset done
                gpsimd.indirect_dma_start(
                    out=TT.ap()[0:rows, c : c + 1],
                    out_offset=None,
                    in_=tags_d[:],
                    in_offset=bass.IndirectOffsetOnAxis(
                        ap=gidx_sb.ap()[0:rows, c : c + 1], axis=0
                    ),
                ).then_inc(tcol[c], 16)

        @block.vector
        def _(vector):
            def chain(instr):
                instr.then_inc(vc, 1)
                return nxt()

            def W():
                vector.wait_ge(vc, vn["n"])

            # 1: keep the unwritten tail of the last gather column defined
            chain(vector.memset(TT.ap()[:, C - 1 : C], 0.0))
            # 2-3: additive pair mask BIGM = BIG * (1 - pv_p*pv_q*blk)
            vector.wait_ge(tc, T_PVT)
            chain(vector.scalar_tensor_tensor(
                out=pm.ap(), in0=pvT, scalar=pvnb_ap, in1=bmask,
                op0=Alu.mult, op1=Alu.mult))
            W()
            i = chain(vector.tensor_scalar(
                out=me.ap(), in0=pm.ap(), scalar1=BIG, scalar2=None,
                op0=Alu.add))
            assert i == V_BIGM
            # per-person stats straight from PSUM
            vector.wait_ge(tc, T_MM(C - 1))
            W()
            i = chain(vector.tensor_mul(mu.ap(), fmup[:, 0:1], rc_ap))
            assert i == V_MU
            W()
            i = chain(vector.tensor_mul(nmu.ap(), fmup[:, 0:1], negrc_ap))
            assert i == V_NMU
            W()
            chain(vector.tensor_copy(t2s.ap(), fmup[:, 1:2]))
            W()
            chain(vector.scalar_tensor_tensor(
                out=negp.ap(), in0=fmup[:, 0:1], scalar=mu.ap(), in1=t2s.ap(),
                op0=Alu.mult, op1=Alu.subtract))
            W()
            i = chain(vector.tensor_scalar(
                out=X.ap()[:, 1:2], in0=negp.ap(), scalar1=rc_ap, scalar2=-1.0,
                op0=Alu.mult, op1=Alu.mult))
            assert i == V_X1
            # epilogue straight from fin PSUM (rd already includes the 0.5)
            vector.wait_ge(tc, T_FIN)
            W()
            i = chain(vector.tensor_scalar(
                out=res.ap()[:, 0:1], in0=fin[:, 0:1], scalar1=nim_ap,
                scalar2=rd_ap, op0=Alu.subtract, op1=Alu.mult))
            assert i == V_RES

        @block.tensor
        def _(tensor):
            tensor.wait_ge(c_sem, 16)
            tensor.transpose(
                out=pvT, in_=pv_ap.to_broadcast([P, P]), identity=ident
            ).then_inc(tc, 1)
            # preload muT's PSUM bank with the (symmetric) additive mask;
            # the mu transpose later accumulates on top of it
            tensor.wait_ge(vc, V_BIGM)
            tensor.matmul(
                out=muT, lhsT=me.ap(), rhs=ident, is_transpose=True,
                start=True, stop=False, skip_group_check=True,
            ).then_inc(tc, 1)
            for c in range(C):
                tensor.wait_ge(sc, S_SQ(c))
                tensor.matmul(
                    out=fmup,
                    lhsT=c_sb.ap()[:, 128 * c : 128 * (c + 1)],
                    rhs=TT.ap()[:, c : 2 * C : C],
                    start=(c == 0),
                    stop=(c == C - 1),
                    skip_group_check=True,
                ).then_inc(tc, 1)
            tensor.wait_ge(vc, V_MU)
            tensor.matmul(
                out=muT, lhsT=mu.ap().to_broadcast([P, P]), rhs=ident,
                is_transpose=True, start=False, stop=True,
                skip_group_check=True,
            ).then_inc(tc, 1)
            tensor.wait_ge(sc, S_EXP)
            tensor.wait_ge(vc, V_X1)
            tensor.matmul(
                out=fin, lhsT=binds, rhs=X.ap(), start=True, stop=True
            ).then_inc(tc, 1)

        @block.scalar
        def _(scalar):
            scalar.wait_ge(c_sem, 16)
            scalar.activation(warm.ap(), c_sb.ap()[0:1, 0:1], Act.Exp)
            for c in range(C):
                scalar.wait_ge(tcol[c], 16)
                scalar.activation(
                    TT.ap()[:, C + c : C + c + 1],
                    TT.ap()[:, c : c + 1],
                    Act.Square,
                ).then_inc(sc, 1)
            scalar.wait_ge(tc, T_MUT)
            scalar.wait_ge(vc, V_NMU)
            scalar.activation(
                d2.ap(), muT, Act.Square, bias=nmu.ap()
            ).then_inc(sc, 1)
            scalar.wait_ge(sc, S_D2)  # d2 write visible before Exp reads it
            scalar.activation(
                e.ap(), d2.ap(), Act.Exp, scale=-1.0, accum_out=X.ap()[:, 0:1]
            ).then_inc(sc, 1)
            # pull output in parallel with res0 on DVE
            scalar.wait_ge(tc, T_FIN)
            scalar.activation(
                res.ap()[:, 1:2], fin[:, 1:2], Act.Copy, scale=rn_ap
            ).then_inc(sc, 1)

    ctx.close()
    _hoist_input_dmas(nc)
    nc.compile()
    return nc


def _hoist_input_dmas(nc):
    """Move the two input DMAs from sync's body block into the entry block,
    right after sync's TPB-base preamble, so the gidx/consts loads overlap
    the remaining engine boot instead of waiting for the entry barrier."""
    f = nc.m.functions[0]
    b0 = f.blocks[0]
    bsync = next(b for b in f.blocks if "_SP_" in b.name)
    assert type(bsync.instructions[0]).__name__ == "InstDMACopy"
    assert type(bsync.instructions[1]).__name__ == "InstDMACopy"
    dmas = [bsync.instructions[0], bsync.instructions[1]]
    rest = list(bsync.instructions)[2:]
    bsync.set_instructions_from_list(rest) if hasattr(
        bsync, "set_instructions_from_list"
    ) else None
    if not hasattr(bsync, "set_instructions_from_list"):
        del bsync.instructions[0]
        del bsync.instructions[0]
    sp = mybir.EngineType.SP
    entry = list(b0.instructions)
    pos = min(i for i, inst in enumerate(entry) if inst.engine == sp)
    b0.instructions[pos:pos] = dmas


def _prepare_in_maps_c(tags, joints, C):
    RC0, NRC0, PV0, PVN0, NI0, RD0, RN0, BM0, BI0, ID0, CC = _consts_layout(C)
    tags = np.ascontiguousarray(tags, dtype=np.float32).reshape(N, KHW)
    joints = np.asarray(joints)
    idx = joints[..., 0].astype(np.int64)
    visb = joints[..., 1] > 0

    blkid = np.repeat(np.arange(IPC), M)
    bmask = np.zeros((P, P), np.float32)
    bmask[:PPI, :PPI] = (blkid[:, None] == blkid[None, :]).astype(np.float32)
    binds = np.zeros((P, IPC), np.float32)
    for i in range(IPC):
        binds[i * M : (i + 1) * M, i] = 1.0
    ident = np.eye(P, dtype=np.float32)

    in_maps = []
    for cidx in range(NCORES):
        sl = slice(cidx * IPC, (cidx + 1) * IPC)
        vb = visb[sl].reshape(PPI, K)
        cnt = vb.sum(axis=1).astype(np.float32)
        gfull = (
            np.arange(IPC, dtype=np.int64)[:, None, None] * KHW + idx[sl]
        ).reshape(PPI, K)
        pp, kk = np.nonzero(vb)
        vals = gfull[pp, kk]
        S = len(pp)
        if S > 128 * C:
            return None
        s = np.arange(S)
        rows, cols = s % P, s // P
        gidxC = np.zeros((P, C), np.int32)
        gidxC[rows, cols] = vals.astype(np.int32)
        lhs = np.zeros((P, C * P), np.float32)
        lhs[rows, cols * P + pp] = 1.0

        pv = (cnt > 0).astype(np.float32)
        nim = pv.reshape(IPC, M).sum(axis=1)  # [IPC]
        consts = np.zeros((P, CC), np.float32)
        consts[:, :RC0] = lhs
        consts[:PPI, RC0] = 1.0 / np.maximum(cnt, 1.0)
        consts[PPI:, RC0] = 1.0
        consts[:, NRC0] = -consts[:, RC0]
        consts[:PPI, PV0] = pv
        consts[:PPI, PVN0] = -BIG * pv
        consts[:IPC, NI0] = nim
        consts[:IPC, RD0] = (0.5 / np.maximum((nim - 1.0) * nim, 1.0)).astype(
            np.float32
        )
        consts[:IPC, RN0] = (1.0 / np.maximum(nim, 1.0)).astype(np.float32)
        consts[:, BM0 : BM0 + P] = bmask
        consts[:, BI0 : BI0 + IPC] = binds
        consts[:, ID0 : ID0 + P] = ident
        in_maps.append(
            {
                "tags": tags[sl].reshape(IPC * KHW, 1),
                "gidx": gidxC,
                "consts": consts,
            }
        )
    return in_maps


_CACHE: dict = {}


def _get_nc(C, R):
    if (C, R) not in _CACHE:
        _CACHE[(C, R)] = _build_nc(C, R)
    return _CACHE[(C, R)]


def _needed_CR(joints):
    visb = np.asarray(joints)[..., 1] > 0
    smax = 0
    for c in range(NCORES):
        S = int(visb[c * IPC : (c + 1) * IPC].sum())
        smax = max(smax, S)
    C = min(max(1, math.ceil(smax / P)), K)
    # last-column fill, rounded up to 32 rows to bound NEFF-cache variants
    R = smax - (C - 1) * P
    R = min(P, max(32, 32 * math.ceil(R / 32)))
    return C, R


def _run(tags, joints, trace=False, **kwargs):
    tags = np.asarray(tags)
    joints = np.asarray(joints)
    C, R = _needed_CR(joints)
    nc = _get_nc(C, R)
    in_maps = _prepare_in_maps_c(tags, joints, C)
    assert in_maps is not None
    res = run_bass_kernel_spmd(
        nc, in_maps, core_ids=list(range(NCORES)), trace=trace, **kwargs
    )
    outs = np.concatenate(
        [np.asarray(res.results[i]["out"]) for i in range(NCORES)], axis=0
    )  # [N, 2]
    pushes = np.ascontiguousarray(outs[:, 0], dtype=np.float32)
    pulls = np.ascontiguousarray(outs[:, 1], dtype=np.float32)
    return (pushes, pulls), res


def kernel(tags, joints):
    (pushes, pulls), _ = _run(tags, joints, trace=False)
    return pushes, pulls
